# revision 7
# baseline (speedup 1.0000x reference)
"""Boundaries-loss kernel for 8 Trainium2 NeuronCores.

Computes: mean_b mean_s( min_v ||bds[b, idx[s], :3] - verts[b, v]||^2 * mask[b, idx[s]] )

Strategy (data-parallel over batch, one batch element per core):
  dist(s, v) = ||b_s||^2 + (||v||^2 - 2<b_s, v>)
  - One matmul with homogeneous K rows produces the full dist(s, v); fp32
    factors are split into three bf16 parts so the PE runs at bf16 rate with
    ~fp32 accuracy (K=24; PE cycles depend only on the moving free dim).
  - Vert tiles are padded to the full 512-wide PSUM bank (pad columns repeat
    vert 0), so every bank is 100% valid and all drain views are plain 2D.
  - Drain (the bottleneck): DVE can ingest PSUM at 1 elem/lane/cycle, ACT at
    1 elem/lane/cycle, and GpSimd can reduce fp16 in SBUF.  Per 5-quad
    s-tile the work is split so all three engines run near-saturated:
    ACT casts ~2 quads to fp16, DVE min-reduces the rest directly from PSUM
    (fused with the casted quads via tensor_tensor_scan where possible),
    Pool folds leftover casted quads.  This keeps per-s-tile drain close to
    the warm PE matmul time, so the PE avoids the 3.4us HAM idle window.
  - Host-side exactness-preserving compaction: samples with mask 0 are
    dropped, and duplicate indices are folded into integer count-weights.
"""

import os
import sys
from contextlib import ExitStack

import numpy as np

for _p in ("/opt/trn_rl_repo", "/root/.axon_site/_ro/trn_rl_repo"):
    if os.path.isdir(_p) and _p not in sys.path:
        sys.path.append(_p)

import ml_dtypes

BT, NV, NB, NS = 8, 10000, 16384, 4096
VT = 512              # vert tile = full PSUM bank
NTV = 20              # number of vert tiles
NVP = NTV * VT        # padded vert count (10240; pad columns repeat vert 0)
K = 24                # 3 coords x 6 part-pairs + 3 sq_v rows + 3 sq_b rows

# Part-index pairs (i, j) kept from (b0+b1+b2)*(w0+w1+w2); dropped terms are
# O(2^-27) relative.
_PAIRS = [(0, 0), (0, 1), (1, 0), (0, 2), (2, 0), (1, 1)]

_BF16 = ml_dtypes.bfloat16

_COMPILED = {}
_LAST_EXEC_NS = None  # set when BOUNDARIES_TRACE=1

# Drain mode: "scan" fuses PSUM quad + casted quad per DVE op via
# tensor_tensor_scan; "ttr" uses tensor_reduce from PSUM + TTR folds in SBUF.
_DRAIN = os.environ.get("BNDS_DRAIN", "scan")


def _bf16_split3(x):
    """x (fp32) -> three bf16 arrays whose fp32 sum matches x to ~2^-27 rel."""
    p0 = x.astype(_BF16)
    r = x - p0.astype(np.float32)
    p1 = r.astype(_BF16)
    r = r - p1.astype(np.float32)
    p2 = r.astype(_BF16)
    return p0, p1, p2


def _build_program(S, drain):
    """Build the per-core SPMD program for S compacted samples (S % 128 == 0)."""
    import concourse.bass as bass  # noqa: F401  (registers engine methods)
    import concourse.tile as tile
    from concourse import bacc, mybir

    T = S // 128
    dt = mybir.dt
    fmin = mybir.AluOpType.min
    nc = bacc.Bacc(
        "TRN2",
        target_bir_lowering=False,
        debug=False,
        enable_asserts=False,
        num_devices=BT,
    )

    QB = 4  # PSUM banks per quad
    NQ = NTV // QB  # quads per s-tile
    QW = QB * VT  # elements per quad (2048)

    lhsT = nc.dram_tensor("lhsT", [K, S], dt.bfloat16, kind="ExternalInput").ap()
    rhs = nc.dram_tensor("rhs", [K, NVP], dt.bfloat16, kind="ExternalInput").ap()
    msk = nc.dram_tensor("msk", [128, T], dt.float32, kind="ExternalInput").ap()
    out = nc.dram_tensor("out", [128, 1], dt.float32, kind="ExternalOutput").ap()

    with tile.TileContext(nc) as tc, ExitStack() as ctx:
        const = ctx.enter_context(tc.tile_pool(name="const", bufs=1))
        psum = ctx.enter_context(tc.tile_pool(name="psum", bufs=2, space="PSUM"))
        cols_pool = ctx.enter_context(tc.tile_pool(name="cols", bufs=4))
        accs = ctx.enter_context(tc.tile_pool(name="accs", bufs=6))

        # Load order matters: the first s-tile's matmuls need only
        # lhsT[:, 0:128] and the first rhs quad — land those first so PE
        # starts ~8us earlier; the bulk loads stream in behind them.
        lhsT_sb = const.tile([K, S], dt.bfloat16)
        rhs_sb = const.tile([K, NVP], dt.bfloat16)
        nc.sync.dma_start(out=lhsT_sb[:, 0:128], in_=lhsT[:, 0:128])
        nc.sync.dma_start(out=rhs_sb[:, 0:QW], in_=rhs[:, 0:QW])
        if S > 128:
            nc.sync.dma_start(out=lhsT_sb[:, 128:S], in_=lhsT[:, 128:S])
        for c in range(1, NQ):
            nc.sync.dma_start(out=rhs_sb[:, c * QW : (c + 1) * QW], in_=rhs[:, c * QW : (c + 1) * QW])
        msk_sb = const.tile([128, T], dt.float32)
        nc.sync.dma_start(out=msk_sb[:], in_=msk)
        mins = const.tile([128, T], dt.float32)

        for t in range(T):
            lw = lhsT_sb[:, t * 128 : (t + 1) * 128]

            if drain == "scan":
                # Per s-tile: ACT casts quads 0 and 2 (plus half of quad 4)
                # to fp16; DVE scans absorb each remaining PSUM quad together
                # with one casted quad, chaining the running min through
                # `initial`.  Engine work per s-tile: ACT ~5.9us, DVE ~6.0us.
                chain = None
                cks = {}
                for q in range(NQ):
                    pq = psum.tile([128, QB * VT], dt.float32, tag="quad")
                    for i in range(QB):
                        v0 = (q * QB + i) * VT
                        nc.tensor.matmul(
                            pq[:, i * VT : (i + 1) * VT], lw, rhs_sb[:, v0 : v0 + VT]
                        )
                    if q in (0, 2):
                        ck = cols_pool.tile([128, QW], dt.float16, tag="chunk")
                        nc.scalar.copy(ck[:], pq[:, 0:QW])
                        cks[q] = ck
                    elif q in (1, 3):
                        scr = cols_pool.tile([128, QW], dt.float32, tag="scr")
                        nc.vector.tensor_tensor_scan(
                            scr[:], pq[:, 0:QW], cks[q - 1][:],
                            initial=3.0e38 if chain is None else chain,
                            op0=fmin, op1=fmin,
                        )
                        chain = scr[:, QW - 1 : QW]
                    else:  # q == 4: split — ACT casts 2 banks, DVE scans 2
                        ckh = cols_pool.tile([128, QW // 2], dt.float16, tag="ckh")
                        nc.scalar.copy(ckh[:], pq[:, 0 : QW // 2])
                        scr = cols_pool.tile([128, QW // 2], dt.float32, tag="scrh")
                        nc.vector.tensor_tensor_scan(
                            scr[:], pq[:, QW // 2 : QW], ckh[:],
                            initial=3.0e38 if chain is None else chain,
                            op0=fmin, op1=fmin,
                        )
                        chain = scr[:, QW // 2 - 1 : QW // 2]
                nc.vector.tensor_tensor(
                    out=mins[:, t : t + 1], in0=chain, in1=chain, op=fmin
                )
            else:  # "ttr": DVE tensor_reduce from PSUM + SBUF TTR folds
                stage = accs.tile([128, 4], dt.float32, tag="stage", name="stage")
                ncol = 0
                cks = []
                for q in range(NQ):
                    pq = psum.tile([128, QB * VT], dt.float32, tag="quad")
                    for i in range(QB):
                        v0 = (q * QB + i) * VT
                        nc.tensor.matmul(
                            pq[:, i * VT : (i + 1) * VT], lw, rhs_sb[:, v0 : v0 + VT]
                        )
                    if q in (1, 3):
                        ck = cols_pool.tile([128, QW], dt.float16, tag="chunk")
                        nc.scalar.copy(ck[:], pq[:, 0:QW])
                        cks.append(ck)
                    else:
                        nc.vector.tensor_reduce(
                            stage[:, ncol : ncol + 1], pq[:, 0:QW],
                            axis=mybir.AxisListType.X, op=fmin,
                        )
                        ncol += 1
                scr = cols_pool.tile([128, QW], dt.float16, tag="scr16")
                nc.vector.tensor_tensor_reduce(
                    scr[:], cks[0][:], cks[1][:], scale=1.0, scalar=3.0e38,
                    op0=fmin, op1=fmin,
                    accum_out=stage[:, ncol : ncol + 1],
                )
                ncol += 1
                nc.vector.tensor_reduce(
                    mins[:, t : t + 1], stage[:, 0:ncol],
                    axis=mybir.AxisListType.X, op=fmin,
                )

        masked = const.tile([128, T], dt.float32)
        nc.vector.tensor_mul(masked[:], mins[:], msk_sb[:])
        col = const.tile([128, 1], dt.float32)
        nc.vector.tensor_reduce(
            col[:], masked[:], axis=mybir.AxisListType.X, op=mybir.AluOpType.add
        )
        nc.sync.dma_start(out=out, in_=col[:])

    nc.compile()
    return nc


def _prep_core_inputs(verts_b, coords_b, w_b, S):
    """Host-side layout prep for one batch element / core.

    verts_b  [NV, 3] fp32, coords_b [na, 3] fp32 (compacted unique samples),
    w_b [na] fp32 weights (count * mask).  Returns the DRAM input map.
    """
    T = S // 128
    na = coords_b.shape[0]

    bpad = np.zeros((S, 3), dtype=np.float32)
    bpad[:na] = coords_b
    wpad = np.zeros((S,), dtype=np.float32)
    wpad[:na] = w_b
    sqb = np.sum(bpad * bpad, axis=-1, dtype=np.float32)

    b_parts = _bf16_split3(bpad)  # each [S, 3]

    vp = np.concatenate([verts_b, np.broadcast_to(verts_b[0:1], (NVP - NV, 3))])
    w = (-2.0 * vp).astype(np.float32)  # [NVP, 3]
    sqv = np.sum(vp * vp, axis=-1, dtype=np.float32)  # [NVP]
    w_parts = _bf16_split3(w)
    s_parts = _bf16_split3(sqv)

    lhsT = np.empty((K, S), dtype=_BF16)
    rhs = np.empty((K, NVP), dtype=_BF16)
    for d in range(3):
        for r, (i, j) in enumerate(_PAIRS):
            lhsT[6 * d + r] = b_parts[i][:, d]
            rhs[6 * d + r] = w_parts[j][:, d]
    for j in range(3):
        lhsT[18 + j] = np.ones((S,), dtype=_BF16)
        rhs[18 + j] = s_parts[j]
    sqb_parts = _bf16_split3(sqb)
    for j in range(3):
        lhsT[21 + j] = sqb_parts[j]
        rhs[21 + j] = np.ones((NVP,), dtype=_BF16)

    return {
        "lhsT": np.ascontiguousarray(lhsT),
        "rhs": np.ascontiguousarray(rhs),
        "msk": np.ascontiguousarray(wpad.reshape(T, 128).T),
    }


def _prepare_all(verts, bds, indices):
    verts = np.asarray(verts, dtype=np.float32)
    bds = np.asarray(bds, dtype=np.float32)
    idx = np.asarray(indices).astype(np.int64)

    # Fold duplicate indices into count-weights (exact: duplicates share the
    # same min distance, so their contributions sum to count * mask * min).
    uniq, counts = np.unique(idx, return_counts=True)

    coords = bds[:, uniq, :3]           # [BT, nu, 3]
    wgt = bds[:, uniq, 3] * counts[None, :].astype(np.float32)  # [BT, nu]

    active = [np.nonzero(wgt[b] != 0.0)[0] for b in range(BT)]
    max_act = max(len(a) for a in active)
    if max_act == 0:
        return None, None
    S = ((max_act + 127) // 128) * 128

    in_maps = [
        _prep_core_inputs(verts[b], coords[b][active[b]], wgt[b][active[b]], S)
        for b in range(BT)
    ]
    return S, in_maps


def _ensure_ntff_hook():
    """Register the NTFF profile hook bass_utils expects under axon.

    This container's ``antenv`` lacks ``axon_hooks``; build the equivalent
    from the boot helper so trace=True can capture neuron-profile output.
    Only used by the local test harness (BOUNDARIES_TRACE=1).
    """
    import types

    try:
        from antenv.axon_hooks import get_axon_ntff_profile_hook  # noqa: F401

        return True
    except ImportError:
        pass
    try:
        import antenv
        from trn_agent_boot.trn_boot import _ntff_profile_via_ctypes

        hook = _ntff_profile_via_ctypes("/opt/axon/libaxon_pjrt.so")
        if hook is None:
            return False
        mod = types.ModuleType("antenv.axon_hooks")
        mod.get_axon_ntff_profile_hook = lambda: hook
        mod.set_axon_ntff_profile_hook = lambda h: None
        sys.modules["antenv.axon_hooks"] = mod
        antenv.axon_hooks = mod
        return True
    except Exception:
        return False


def kernel(verts, bds, pix_to_face, indices):
    global _LAST_EXEC_NS
    S, in_maps = _prepare_all(verts, bds, indices)
    if S is None:
        return np.float32(0.0)

    key = (S, _DRAIN)
    if key not in _COMPILED:
        _COMPILED[key] = _build_program(S, _DRAIN)
    nc = _COMPILED[key]

    from concourse import bass_utils

    trace = os.environ.get("BOUNDARIES_TRACE", "0") == "1" and _ensure_ntff_hook()
    if trace:
        # Local profiling only: skip the artifact-bucket upload.
        bass_utils.upload_artifacts = lambda tmpdir: "local://unused"

    try:
        res = bass_utils.run_bass_kernel_spmd(
            nc, in_maps, core_ids=list(range(BT)), trace=trace
        )
    except Exception:
        if not trace:
            raise
        res = bass_utils.run_bass_kernel_spmd(
            nc, in_maps, core_ids=list(range(BT)), trace=False
        )
    _LAST_EXEC_NS = res.exec_time_ns

    total = sum(
        float(np.sum(res.results[b]["out"].astype(np.float64))) for b in range(BT)
    )
    return np.float32(total / (NS * BT))


if __name__ == "__main__":
    # Quick self-check against a local numpy reference on random data.
    rng = np.random.default_rng(0)
    verts = rng.standard_normal((BT, NV, 3), dtype=np.float32)
    bds = rng.standard_normal((BT, NB, 4), dtype=np.float32)
    bds[..., 3] = (rng.random((BT, NB)) > 0.5).astype(np.float32)
    pix = np.zeros((BT, 256, 256, 1), dtype=np.int32)
    # include duplicates to exercise the dedup path
    idx = rng.integers(0, NB, size=NS).astype(np.int64)

    bv = bds[:, idx, :3]
    bm = bds[:, idx, 3]
    d = (
        np.sum(bv * bv, -1)[:, :, None]
        + np.sum(verts * verts, -1)[:, None, :]
        - 2.0 * np.einsum("bsd,bvd->bsv", bv, verts)
    )
    expected = np.mean(np.min(d, -1) * bm)

    actual = kernel(verts, bds, pix, idx)
    rel = abs(actual - expected) / max(abs(expected), 1e-12)
    print(f"expected={expected:.8f} actual={actual:.8f} rel={rel:.3e}")


# revision 11
# speedup vs baseline: 1.1740x; 1.1740x over previous
"""Boundaries-loss kernel for 8 Trainium2 NeuronCores.

Computes: mean_b mean_s( min_v ||bds[b, idx[s], :3] - verts[b, v]||^2 * mask[b, idx[s]] )

Strategy (data-parallel over batch, one batch element per core):
  dist(s, v) = ||b_s||^2 + (||v||^2 - 2<b_s, v>)
  - One matmul with homogeneous K rows produces the full dist(s, v); fp32
    factors are split into three bf16 parts so the PE runs at bf16 rate with
    ~fp32 accuracy (K=24; PE cycles depend only on the moving free dim).
    Vert tiles fill the whole 512-wide PSUM bank (pad columns repeat verts),
    so drain views are plain 2D.
  - Drain: per 5-quad s-tile, ACT casts 3 quads to fp16 in SBUF, the DVE
    min-reduces the other 2 directly from PSUM, and the casted quads are
    folded by a Pool(GpSimd) elementwise min + DVE fp16 2x tensor_tensor +
    one fp16 reduce.  This splits the 10240-element drain across all three
    programmable engines (~8us each per s-tile) instead of casting
    everything through ACT (the baseline's ~11.6us/s-tile bottleneck), and
    the tighter pace keeps the PE inside its 3.4us HAM window (2.4 GHz).
  - Samples whose mask is exactly 0 are compacted away on the host, and
    duplicate indices fold into integer count-weights (both exact).
"""

import os
import sys
from contextlib import ExitStack

import numpy as np

for _p in ("/opt/trn_rl_repo", "/root/.axon_site/_ro/trn_rl_repo"):
    if os.path.isdir(_p) and _p not in sys.path:
        sys.path.append(_p)

import ml_dtypes

BT, NV, NB, NS = 8, 10000, 16384, 4096
VT = 512              # vert tile = full PSUM bank
NTV = 20              # number of vert tiles
NVP = NTV * VT        # padded vert count (pad columns repeat verts)
K = 24                # 3 coords x 6 part-pairs + 3 sq_v rows + 3 sq_b rows

_PAIRS = [(0, 0), (0, 1), (1, 0), (0, 2), (2, 0), (1, 1)]

_BF16 = ml_dtypes.bfloat16

_COMPILED = {}
_LAST_EXEC_NS = None  # set when BOUNDARIES_TRACE=1

# Use the GpSimd engine for the first fp16 fold (1 = yes).
_POOL = os.environ.get("BNDS_POOL", "1") == "1"


def _bf16_split3(x):
    """x (fp32) -> three bf16 arrays whose fp32 sum matches x to ~2^-27 rel."""
    p0 = x.astype(_BF16)
    r = x - p0.astype(np.float32)
    p1 = r.astype(_BF16)
    r = r - p1.astype(np.float32)
    p2 = r.astype(_BF16)
    return p0, p1, p2


def _build_program(S, use_pool):
    """Build the per-core SPMD program for S compacted samples (S % 128 == 0)."""
    import concourse.bass as bass  # noqa: F401  (registers engine methods)
    import concourse.tile as tile
    from concourse import bacc, mybir

    T = S // 128
    dt = mybir.dt
    fmin = mybir.AluOpType.min
    nc = bacc.Bacc(
        "TRN2",
        target_bir_lowering=False,
        debug=False,
        enable_asserts=False,
        num_devices=BT,
    )

    QB = 4  # PSUM banks per quad
    NQ = NTV // QB  # quads per s-tile
    QW = QB * VT  # elements per quad (2048)

    lhsT = nc.dram_tensor("lhsT", [K, S], dt.bfloat16, kind="ExternalInput").ap()
    rhs = nc.dram_tensor("rhs", [K, NVP], dt.bfloat16, kind="ExternalInput").ap()
    msk = nc.dram_tensor("msk", [128, T], dt.float32, kind="ExternalInput").ap()
    out = nc.dram_tensor("out", [128, 1], dt.float32, kind="ExternalOutput").ap()

    with tile.TileContext(nc) as tc, ExitStack() as ctx:
        const = ctx.enter_context(tc.tile_pool(name="const", bufs=1))
        psum = ctx.enter_context(tc.tile_pool(name="psum", bufs=2, space="PSUM"))
        cols_pool = ctx.enter_context(tc.tile_pool(name="cols", bufs=4))
        accs = ctx.enter_context(tc.tile_pool(name="accs", bufs=4))

        # Land the first s-tile's operands first so the PE starts early.
        lhsT_sb = const.tile([K, S], dt.bfloat16)
        rhs_sb = const.tile([K, NVP], dt.bfloat16)
        nc.sync.dma_start(out=lhsT_sb[:, 0:128], in_=lhsT[:, 0:128])
        nc.sync.dma_start(out=rhs_sb[:, 0:QW], in_=rhs[:, 0:QW])
        if S > 128:
            nc.sync.dma_start(out=lhsT_sb[:, 128:S], in_=lhsT[:, 128:S])
        for c in range(1, NQ):
            nc.sync.dma_start(
                out=rhs_sb[:, c * QW : (c + 1) * QW], in_=rhs[:, c * QW : (c + 1) * QW]
            )
        msk_sb = const.tile([128, T], dt.float32)
        nc.sync.dma_start(out=msk_sb[:], in_=msk)
        mins = const.tile([128, T], dt.float32)

        for t in range(T):
            lw = lhsT_sb[:, t * 128 : (t + 1) * 128]
            stage = accs.tile([128, 4], dt.float32, tag="stage", name="stage")
            ncol = 0
            cks = []
            for q in range(NQ):
                pq = psum.tile([128, QW], dt.float32, tag="quad")
                for i in range(QB):
                    v0 = (q * QB + i) * VT
                    nc.tensor.matmul(
                        pq[:, i * VT : (i + 1) * VT], lw, rhs_sb[:, v0 : v0 + VT]
                    )
                if q in (0, 2, 4):  # ACT casts quads 0, 2, 4 to fp16
                    ck = cols_pool.tile([128, QW], dt.float16, tag="chunk")
                    nc.scalar.copy(ck[:], pq[:, 0:QW])
                    cks.append(ck)
                else:  # DVE min-reduces quads 1, 3 straight from PSUM
                    nc.vector.tensor_reduce(
                        stage[:, ncol : ncol + 1], pq[:, 0:QW],
                        axis=mybir.AxisListType.X, op=fmin,
                    )
                    ncol += 1
            # Fold the three casted quads.
            p1 = cols_pool.tile([128, QW], dt.float16, tag="fold1")
            if use_pool:
                nc.gpsimd.tensor_tensor(out=p1[:], in0=cks[0][:], in1=cks[1][:], op=fmin)
            else:
                nc.vector.tensor_tensor(out=p1[:], in0=cks[0][:], in1=cks[1][:], op=fmin)
            r = cols_pool.tile([128, QW], dt.float16, tag="fold2")
            nc.vector.tensor_tensor(out=r[:], in0=p1[:], in1=cks[2][:], op=fmin)
            r2 = cols_pool.tile([128, QW // 2], dt.float16, tag="fold3")
            if use_pool:
                nc.gpsimd.tensor_tensor(
                    out=r2[:], in0=r[:, 0 : QW // 2], in1=r[:, QW // 2 : QW], op=fmin
                )
                nc.vector.tensor_reduce(
                    stage[:, ncol : ncol + 1], r2[:],
                    axis=mybir.AxisListType.X, op=fmin,
                )
            else:
                nc.vector.tensor_reduce(
                    stage[:, ncol : ncol + 1], r[:],
                    axis=mybir.AxisListType.X, op=fmin,
                )
            ncol += 1
            nc.vector.tensor_reduce(
                mins[:, t : t + 1], stage[:, 0:ncol],
                axis=mybir.AxisListType.X, op=fmin,
            )

        masked = const.tile([128, T], dt.float32)
        nc.vector.tensor_mul(masked[:], mins[:], msk_sb[:])
        col = const.tile([128, 1], dt.float32)
        nc.vector.tensor_reduce(
            col[:], masked[:], axis=mybir.AxisListType.X, op=mybir.AluOpType.add
        )
        nc.sync.dma_start(out=out, in_=col[:])

    nc.compile()
    return nc


def _prep_core_inputs(verts_b, coords_b, w_b, S):
    """Host-side layout prep for one batch element / core."""
    T = S // 128
    na = coords_b.shape[0]

    bpad = np.zeros((S, 3), dtype=np.float32)
    bpad[:na] = coords_b
    wpad = np.zeros((S,), dtype=np.float32)
    wpad[:na] = w_b
    sqb = np.sum(bpad * bpad, axis=-1, dtype=np.float32)

    b_parts = _bf16_split3(bpad)  # each [S, 3]

    vp = np.concatenate([verts_b, np.broadcast_to(verts_b[0:1], (NVP - NV, 3))])
    w = (-2.0 * vp).astype(np.float32)  # [NVP, 3]
    sqv = np.sum(vp * vp, axis=-1, dtype=np.float32)  # [NVP]
    w_parts = _bf16_split3(w)
    s_parts = _bf16_split3(sqv)

    lhsT = np.empty((K, S), dtype=_BF16)
    rhs = np.empty((K, NVP), dtype=_BF16)
    for d in range(3):
        for r, (i, j) in enumerate(_PAIRS):
            lhsT[6 * d + r] = b_parts[i][:, d]
            rhs[6 * d + r] = w_parts[j][:, d]
    for j in range(3):
        lhsT[18 + j] = np.ones((S,), dtype=_BF16)
        rhs[18 + j] = s_parts[j]
    sqb_parts = _bf16_split3(sqb)
    for j in range(3):
        lhsT[21 + j] = sqb_parts[j]
        rhs[21 + j] = np.ones((NVP,), dtype=_BF16)

    return {
        "lhsT": np.ascontiguousarray(lhsT),
        "rhs": np.ascontiguousarray(rhs),
        "msk": np.ascontiguousarray(wpad.reshape(T, 128).T),
    }


def _prepare_all(verts, bds, indices):
    verts = np.asarray(verts, dtype=np.float32)
    bds = np.asarray(bds, dtype=np.float32)
    idx = np.asarray(indices).astype(np.int64)

    # Duplicate indices share a min distance: fold them into count-weights.
    uniq, counts = np.unique(idx, return_counts=True)
    coords = bds[:, uniq, :3]
    wgt = bds[:, uniq, 3] * counts[None, :].astype(np.float32)

    active = [np.nonzero(wgt[b] != 0.0)[0] for b in range(BT)]
    max_act = max(len(a) for a in active)
    if max_act == 0:
        return None, None
    S = ((max_act + 127) // 128) * 128

    in_maps = [
        _prep_core_inputs(verts[b], coords[b][active[b]], wgt[b][active[b]], S)
        for b in range(BT)
    ]
    return S, in_maps


def _ensure_ntff_hook():
    """Register the NTFF profile hook bass_utils expects under axon."""
    import types

    try:
        from antenv.axon_hooks import get_axon_ntff_profile_hook  # noqa: F401

        return True
    except ImportError:
        pass
    try:
        import antenv
        from trn_agent_boot.trn_boot import _ntff_profile_via_ctypes

        hook = _ntff_profile_via_ctypes("/opt/axon/libaxon_pjrt.so")
        if hook is None:
            return False
        mod = types.ModuleType("antenv.axon_hooks")
        mod.get_axon_ntff_profile_hook = lambda: hook
        mod.set_axon_ntff_profile_hook = lambda h: None
        sys.modules["antenv.axon_hooks"] = mod
        antenv.axon_hooks = mod
        return True
    except Exception:
        return False


def kernel(verts, bds, pix_to_face, indices):
    global _LAST_EXEC_NS
    S, in_maps = _prepare_all(verts, bds, indices)
    if S is None:
        return np.float32(0.0)

    key = (S, _POOL)
    if key not in _COMPILED:
        _COMPILED[key] = _build_program(S, _POOL)
    nc = _COMPILED[key]

    from concourse import bass_utils

    trace = os.environ.get("BOUNDARIES_TRACE", "0") == "1" and _ensure_ntff_hook()
    if trace:
        bass_utils.upload_artifacts = lambda tmpdir: "local://unused"

    try:
        res = bass_utils.run_bass_kernel_spmd(
            nc, in_maps, core_ids=list(range(BT)), trace=trace
        )
    except Exception:
        if not trace:
            raise
        res = bass_utils.run_bass_kernel_spmd(
            nc, in_maps, core_ids=list(range(BT)), trace=False
        )
    _LAST_EXEC_NS = res.exec_time_ns

    total = sum(
        float(np.sum(res.results[b]["out"].astype(np.float64))) for b in range(BT)
    )
    return np.float32(total / (NS * BT))


if __name__ == "__main__":
    rng = np.random.default_rng(0)
    verts = rng.standard_normal((BT, NV, 3), dtype=np.float32)
    bds = rng.standard_normal((BT, NB, 4), dtype=np.float32)
    bds[..., 3] = (rng.random((BT, NB)) > 0.5).astype(np.float32)
    pix = np.zeros((BT, 256, 256, 1), dtype=np.int32)
    idx = rng.integers(0, NB, size=NS).astype(np.int64)

    bv = bds[:, idx, :3]
    bm = bds[:, idx, 3]
    d = (
        np.sum(bv * bv, -1)[:, :, None]
        + np.sum(verts * verts, -1)[:, None, :]
        - 2.0 * np.einsum("bsd,bvd->bsv", bv, verts)
    )
    expected = np.mean(np.min(d, -1) * bm)

    actual = kernel(verts, bds, pix, idx)
    rel = abs(actual - expected) / max(abs(expected), 1e-12)
    print(f"expected={expected:.8f} actual={actual:.8f} rel={rel:.3e}")


# revision 14
# speedup vs baseline: 1.2232x; 1.0420x over previous
"""Boundaries-loss kernel for 8 Trainium2 NeuronCores.

Computes: mean_b mean_s( min_v ||bds[b, idx[s], :3] - verts[b, v]||^2 * mask[b, idx[s]] )

Strategy (data-parallel over batch, one batch element per core):
  dist(s, v) = ||b_s||^2 + (||v||^2 - 2<b_s, v>)
  - One matmul with homogeneous K rows produces the full dist(s, v); fp32
    factors are split into three bf16 parts so the PE runs at bf16 rate with
    ~fp32 accuracy (K=24; PE cycles depend only on the moving free dim).
    Vert tiles fill the whole 512-wide PSUM bank (pad columns repeat verts),
    so drain views are plain 2D.
  - Drain: per 5-quad s-tile, ACT casts 3 quads to fp16 in SBUF, the DVE
    min-reduces the other 2 directly from PSUM, and the casted quads are
    folded by a Pool(GpSimd) elementwise min + DVE fp16 2x tensor_tensor +
    one fp16 reduce.  This splits the 10240-element drain across all three
    programmable engines (~8us each per s-tile) instead of casting
    everything through ACT (the baseline's ~11.6us/s-tile bottleneck), and
    the tighter pace keeps the PE inside its 3.4us HAM window (2.4 GHz).
  - Samples whose mask is exactly 0 are compacted away on the host, and
    duplicate indices fold into integer count-weights (both exact).
"""

import os
import sys
from contextlib import ExitStack

import numpy as np

for _p in ("/opt/trn_rl_repo", "/root/.axon_site/_ro/trn_rl_repo"):
    if os.path.isdir(_p) and _p not in sys.path:
        sys.path.append(_p)

import ml_dtypes

BT, NV, NB, NS = 8, 10000, 16384, 4096
VT = 512              # vert tile = full PSUM bank
NTV = 20              # number of vert tiles
NVP = NTV * VT        # padded vert count (pad columns repeat verts)
K = 24                # 3 coords x 6 part-pairs + 3 sq_v rows + 3 sq_b rows

_PAIRS = [(0, 0), (0, 1), (1, 0), (0, 2), (2, 0), (1, 1)]

_BF16 = ml_dtypes.bfloat16

_COMPILED = {}
_LAST_EXEC_NS = None  # set when BOUNDARIES_TRACE=1

# Use the GpSimd engine for the first fp16 fold (1 = yes).
_POOL = os.environ.get("BNDS_POOL", "1") == "1"


def _bf16_split3(x):
    """x (fp32) -> three bf16 arrays whose fp32 sum matches x to ~2^-27 rel."""
    p0 = x.astype(_BF16)
    r = x - p0.astype(np.float32)
    p1 = r.astype(_BF16)
    r = r - p1.astype(np.float32)
    p2 = r.astype(_BF16)
    return p0, p1, p2


def _build_program(S, use_pool):
    """Build the per-core SPMD program for S compacted samples (S % 128 == 0)."""
    import concourse.bass as bass  # noqa: F401  (registers engine methods)
    import concourse.tile as tile
    from concourse import bacc, mybir

    T = S // 128
    dt = mybir.dt
    fmin = mybir.AluOpType.min
    nc = bacc.Bacc(
        "TRN2",
        target_bir_lowering=False,
        debug=False,
        enable_asserts=False,
        num_devices=BT,
    )

    QB = 4  # PSUM banks per quad
    NQ = NTV // QB  # quads per s-tile
    QW = QB * VT  # elements per quad (2048)

    lhsT = nc.dram_tensor("lhsT", [K, S], dt.bfloat16, kind="ExternalInput").ap()
    rhs = nc.dram_tensor("rhs", [K, NVP], dt.bfloat16, kind="ExternalInput").ap()
    msk = nc.dram_tensor("msk", [128, T], dt.float32, kind="ExternalInput").ap()
    out = nc.dram_tensor("out", [128, 1], dt.float32, kind="ExternalOutput").ap()

    with tile.TileContext(nc) as tc, ExitStack() as ctx:
        const = ctx.enter_context(tc.tile_pool(name="const", bufs=1))
        psum = ctx.enter_context(tc.tile_pool(name="psum", bufs=2, space="PSUM"))
        cols_pool = ctx.enter_context(tc.tile_pool(name="cols", bufs=4))
        accs = ctx.enter_context(tc.tile_pool(name="accs", bufs=4))

        # Land the first s-tile's operands first so the PE starts early.
        lhsT_sb = const.tile([K, S], dt.bfloat16)
        rhs_sb = const.tile([K, NVP], dt.bfloat16)
        nc.sync.dma_start(out=lhsT_sb[:, 0:128], in_=lhsT[:, 0:128])
        nc.sync.dma_start(out=rhs_sb[:, 0:QW], in_=rhs[:, 0:QW])
        if S > 128:
            nc.sync.dma_start(out=lhsT_sb[:, 128:S], in_=lhsT[:, 128:S])
        for c in range(1, NQ):
            nc.sync.dma_start(
                out=rhs_sb[:, c * QW : (c + 1) * QW], in_=rhs[:, c * QW : (c + 1) * QW]
            )
        msk_sb = const.tile([128, T], dt.float32)
        nc.sync.dma_start(out=msk_sb[:], in_=msk)
        mins = const.tile([128, T], dt.float32)

        # PE warm-up: ~5us of back-to-back matmuls fires the HAM activity
        # monitor (K=8/8 -> 2.4 GHz).  The steady-state drain keeps PE gaps
        # under the 3.4us re-throttle window afterwards.
        warm = psum.tile([128, QW], dt.float32, tag="quad", name="warm")
        wsink = const.tile([128, 1], dt.float32)
        for _ in range(22):
            nc.tensor.matmul(warm[:, 0:VT], lhsT_sb[:, 0:128], rhs_sb[:, 0:VT])
        nc.vector.tensor_reduce(
            wsink[:], warm[:, 0:VT], axis=mybir.AxisListType.X, op=fmin
        )

        for t in range(T):
            lw = lhsT_sb[:, t * 128 : (t + 1) * 128]
            stage = accs.tile([128, 4], dt.float32, tag="stage", name="stage")
            ncol = 0
            cks = []
            for q in range(NQ):
                pq = psum.tile([128, QW], dt.float32, tag="quad")
                for i in range(QB):
                    v0 = (q * QB + i) * VT
                    nc.tensor.matmul(
                        pq[:, i * VT : (i + 1) * VT], lw, rhs_sb[:, v0 : v0 + VT]
                    )
                if q != 2:  # ACT casts quads 0, 1, 3, 4 to fp16
                    ck = cols_pool.tile([128, QW], dt.float16, tag="chunk")
                    nc.scalar.copy(ck[:], pq[:, 0:QW])
                    cks.append(ck)
                else:  # DVE min-reduces quad 2 straight from PSUM
                    nc.vector.tensor_reduce(
                        stage[:, ncol : ncol + 1], pq[:, 0:QW],
                        axis=mybir.AxisListType.X, op=fmin,
                    )
                    ncol += 1
            # Fold the four casted quads (fp16 tensor_tensor runs in 2x mode).
            p1 = cols_pool.tile([128, QW], dt.float16, tag="fold1")
            nc.vector.tensor_tensor(out=p1[:], in0=cks[0][:], in1=cks[1][:], op=fmin)
            p2 = cols_pool.tile([128, QW], dt.float16, tag="fold2")
            nc.vector.tensor_tensor(out=p2[:], in0=cks[2][:], in1=cks[3][:], op=fmin)
            r = cols_pool.tile([128, QW], dt.float16, tag="fold3")
            nc.vector.tensor_tensor(out=r[:], in0=p1[:], in1=p2[:], op=fmin)
            nc.vector.tensor_reduce(
                stage[:, ncol : ncol + 1], r[:],
                axis=mybir.AxisListType.X, op=fmin,
            )
            ncol += 1
            nc.vector.tensor_reduce(
                mins[:, t : t + 1], stage[:, 0:ncol],
                axis=mybir.AxisListType.X, op=fmin,
            )

        masked = const.tile([128, T], dt.float32)
        nc.vector.tensor_mul(masked[:], mins[:], msk_sb[:])
        col = const.tile([128, 1], dt.float32)
        nc.vector.tensor_reduce(
            col[:], masked[:], axis=mybir.AxisListType.X, op=mybir.AluOpType.add
        )
        nc.sync.dma_start(out=out, in_=col[:])

    nc.compile()
    return nc


def _prep_core_inputs(verts_b, coords_b, w_b, S):
    """Host-side layout prep for one batch element / core."""
    T = S // 128
    na = coords_b.shape[0]

    bpad = np.zeros((S, 3), dtype=np.float32)
    bpad[:na] = coords_b
    wpad = np.zeros((S,), dtype=np.float32)
    wpad[:na] = w_b
    sqb = np.sum(bpad * bpad, axis=-1, dtype=np.float32)

    b_parts = _bf16_split3(bpad)  # each [S, 3]

    vp = np.concatenate([verts_b, np.broadcast_to(verts_b[0:1], (NVP - NV, 3))])
    w = (-2.0 * vp).astype(np.float32)  # [NVP, 3]
    sqv = np.sum(vp * vp, axis=-1, dtype=np.float32)  # [NVP]
    w_parts = _bf16_split3(w)
    s_parts = _bf16_split3(sqv)

    lhsT = np.empty((K, S), dtype=_BF16)
    rhs = np.empty((K, NVP), dtype=_BF16)
    for d in range(3):
        for r, (i, j) in enumerate(_PAIRS):
            lhsT[6 * d + r] = b_parts[i][:, d]
            rhs[6 * d + r] = w_parts[j][:, d]
    for j in range(3):
        lhsT[18 + j] = np.ones((S,), dtype=_BF16)
        rhs[18 + j] = s_parts[j]
    sqb_parts = _bf16_split3(sqb)
    for j in range(3):
        lhsT[21 + j] = sqb_parts[j]
        rhs[21 + j] = np.ones((NVP,), dtype=_BF16)

    return {
        "lhsT": np.ascontiguousarray(lhsT),
        "rhs": np.ascontiguousarray(rhs),
        "msk": np.ascontiguousarray(wpad.reshape(T, 128).T),
    }


def _prepare_all(verts, bds, indices):
    verts = np.asarray(verts, dtype=np.float32)
    bds = np.asarray(bds, dtype=np.float32)
    idx = np.asarray(indices).astype(np.int64)

    # Duplicate indices share a min distance: fold them into count-weights.
    uniq, counts = np.unique(idx, return_counts=True)
    coords = bds[:, uniq, :3]
    wgt = bds[:, uniq, 3] * counts[None, :].astype(np.float32)

    active = [np.nonzero(wgt[b] != 0.0)[0] for b in range(BT)]
    max_act = max(len(a) for a in active)
    if max_act == 0:
        return None, None
    S = ((max_act + 127) // 128) * 128

    in_maps = [
        _prep_core_inputs(verts[b], coords[b][active[b]], wgt[b][active[b]], S)
        for b in range(BT)
    ]
    return S, in_maps


def _ensure_ntff_hook():
    """Register the NTFF profile hook bass_utils expects under axon."""
    import types

    try:
        from antenv.axon_hooks import get_axon_ntff_profile_hook  # noqa: F401

        return True
    except ImportError:
        pass
    try:
        import antenv
        from trn_agent_boot.trn_boot import _ntff_profile_via_ctypes

        hook = _ntff_profile_via_ctypes("/opt/axon/libaxon_pjrt.so")
        if hook is None:
            return False
        mod = types.ModuleType("antenv.axon_hooks")
        mod.get_axon_ntff_profile_hook = lambda: hook
        mod.set_axon_ntff_profile_hook = lambda h: None
        sys.modules["antenv.axon_hooks"] = mod
        antenv.axon_hooks = mod
        return True
    except Exception:
        return False


def kernel(verts, bds, pix_to_face, indices):
    global _LAST_EXEC_NS
    S, in_maps = _prepare_all(verts, bds, indices)
    if S is None:
        return np.float32(0.0)

    key = (S, _POOL)
    if key not in _COMPILED:
        _COMPILED[key] = _build_program(S, _POOL)
    nc = _COMPILED[key]

    from concourse import bass_utils

    trace = os.environ.get("BOUNDARIES_TRACE", "0") == "1" and _ensure_ntff_hook()
    if trace:
        bass_utils.upload_artifacts = lambda tmpdir: "local://unused"

    try:
        res = bass_utils.run_bass_kernel_spmd(
            nc, in_maps, core_ids=list(range(BT)), trace=trace
        )
    except Exception:
        if not trace:
            raise
        res = bass_utils.run_bass_kernel_spmd(
            nc, in_maps, core_ids=list(range(BT)), trace=False
        )
    _LAST_EXEC_NS = res.exec_time_ns

    total = sum(
        float(np.sum(res.results[b]["out"].astype(np.float64))) for b in range(BT)
    )
    return np.float32(total / (NS * BT))


if __name__ == "__main__":
    rng = np.random.default_rng(0)
    verts = rng.standard_normal((BT, NV, 3), dtype=np.float32)
    bds = rng.standard_normal((BT, NB, 4), dtype=np.float32)
    bds[..., 3] = (rng.random((BT, NB)) > 0.5).astype(np.float32)
    pix = np.zeros((BT, 256, 256, 1), dtype=np.int32)
    idx = rng.integers(0, NB, size=NS).astype(np.int64)

    bv = bds[:, idx, :3]
    bm = bds[:, idx, 3]
    d = (
        np.sum(bv * bv, -1)[:, :, None]
        + np.sum(verts * verts, -1)[:, None, :]
        - 2.0 * np.einsum("bsd,bvd->bsv", bv, verts)
    )
    expected = np.mean(np.min(d, -1) * bm)

    actual = kernel(verts, bds, pix, idx)
    rel = abs(actual - expected) / max(abs(expected), 1e-12)
    print(f"expected={expected:.8f} actual={actual:.8f} rel={rel:.3e}")


# revision 15
# speedup vs baseline: 1.5178x; 1.2408x over previous
"""Boundaries-loss kernel for 8 Trainium2 NeuronCores.

Computes: mean_b mean_s( min_v ||bds[b, idx[s], :3] - verts[b, v]||^2 * mask[b, idx[s]] )

Strategy (data-parallel over batch, one batch element per core):
  dist(s, v) = ||b_s||^2 + (||v||^2 - 2<b_s, v>)
  - The full dist(s, v) is produced by one matmul with homogeneous K rows
    (coords, ||v||^2, and ||b||^2 rows).  To run the PE at bf16 rate with
    ~fp32 accuracy, every fp32 factor is split into three bf16 parts
    (hi/mid/lo) and the significant part-products map to extra contraction
    rows (K=24).  PE cycles depend only on the moving free dim, so the
    extra K rows are free.
  - PSUM drain / min-reduction: the scalar engine casts each 4-bank quad to
    fp16 in SBUF (distances are well-conditioned in fp16 since ||b||^2 is
    folded into the matmul); the DVE chains 2x-mode fp16 tensor_tensor mins
    and one final 1x reduce per sample tile.  DVE-only fp32 reduce from
    PSUM is the 1 elem/lane/cycle wall; this splits the drain across ACT
    and DVE.
  - Samples whose mask is exactly 0 contribute exactly 0 to the loss, so they
    are compacted away on the host (exact for any mask values).
"""

import os
import sys
from contextlib import ExitStack

import numpy as np

for _p in ("/opt/trn_rl_repo", "/root/.axon_site/_ro/trn_rl_repo"):
    if os.path.isdir(_p) and _p not in sys.path:
        sys.path.append(_p)

import ml_dtypes

BT, NV, NB, NS = 8, 10000, 16384, 4096
VT = 500              # vert tile (matmul free dim; 10000 = 20 x 500, no padding)
BANK = 512            # PSUM bank stride in fp32 elements
NTV = 20              # number of vert tiles
K = 24                # 3 coords x 6 part-pairs + 3 sq_v rows + 3 sq_b rows

# Part-index pairs (i, j) kept from (b0+b1+b2)*(w0+w1+w2); dropped terms are
# O(2^-27) relative.
_PAIRS = [(0, 0), (0, 1), (1, 0), (0, 2), (2, 0), (1, 1)]

_BF16 = ml_dtypes.bfloat16

_COMPILED = {}        # (S,) -> (nc, names) cache
_LAST_EXEC_NS = None  # set when BOUNDARIES_TRACE=1


def _bf16_split3(x):
    """x (fp32) -> three bf16 arrays whose fp32 sum matches x to ~2^-27 rel."""
    p0 = x.astype(_BF16)
    r = x - p0.astype(np.float32)
    p1 = r.astype(_BF16)
    r = r - p1.astype(np.float32)
    p2 = r.astype(_BF16)
    return p0, p1, p2


def _build_program(S):
    """Build the per-core SPMD program for S compacted samples (S % 128 == 0)."""
    import concourse.bass as bass  # noqa: F401  (registers engine methods)
    import concourse.tile as tile
    from concourse import bacc, mybir

    T = S // 128
    dt = mybir.dt
    nc = bacc.Bacc(
        "TRN2",
        target_bir_lowering=False,
        debug=False,
        enable_asserts=False,
        num_devices=BT,
    )

    QB = 4  # PSUM banks per reduce quad
    NQ = NTV // QB  # quads per s-tile
    lhsT = nc.dram_tensor("lhsT", [K, S], dt.bfloat16, kind="ExternalInput").ap()
    rhs = nc.dram_tensor("rhs", [K, NV], dt.bfloat16, kind="ExternalInput").ap()
    msk = nc.dram_tensor("msk", [128, T], dt.float32, kind="ExternalInput").ap()
    out = nc.dram_tensor("out", [128, 1], dt.float32, kind="ExternalOutput").ap()

    with tile.TileContext(nc) as tc, ExitStack() as ctx:
        const = ctx.enter_context(tc.tile_pool(name="const", bufs=1))
        psum = ctx.enter_context(tc.tile_pool(name="psum", bufs=2, space="PSUM"))
        cols_pool = ctx.enter_context(tc.tile_pool(name="cols", bufs=6))
        accs = ctx.enter_context(tc.tile_pool(name="accs", bufs=2))

        # Load order matters: the first s-tile's matmuls need only
        # lhsT[:, 0:128] and the first rhs quad — land those first so PE/ACT
        # start ~8us earlier; the bulk loads stream in behind them.
        lhsT_sb = const.tile([K, S], dt.bfloat16)
        rhs_sb = const.tile([K, NV], dt.bfloat16)
        nc.sync.dma_start(out=lhsT_sb[:, 0:128], in_=lhsT[:, 0:128])
        nc.sync.dma_start(out=rhs_sb[:, 0 : QB * VT], in_=rhs[:, 0 : QB * VT])
        if S > 128:
            nc.sync.dma_start(out=lhsT_sb[:, 128:S], in_=lhsT[:, 128:S])
        for c in range(1, NQ):
            lo, hi = c * QB * VT, min((c + 1) * QB * VT, NV)
            nc.sync.dma_start(out=rhs_sb[:, lo:hi], in_=rhs[:, lo:hi])
        msk_sb = const.tile([128, T], dt.float32)
        nc.sync.dma_start(out=msk_sb[:], in_=msk)
        mins = const.tile([128, T], dt.float32)

        for t in range(T):
            lw = lhsT_sb[:, t * 128 : (t + 1) * 128]
            running = None
            for q in range(NQ):
                pq = psum.tile([128, QB * BANK], dt.float32, tag="quad")
                for i in range(QB):
                    v0 = (q * QB + i) * VT
                    nc.tensor.matmul(
                        pq[:, i * BANK : i * BANK + VT], lw, rhs_sb[:, v0 : v0 + VT]
                    )
                pq_view = pq[:].rearrange("p (b v) -> p b v", b=QB)[:, :, 0:VT]
                # ACT casts the quad to bf16 in SBUF; DVE min-chains at 2x.
                ck = cols_pool.tile([128, QB * VT], dt.float16, tag="chunk")
                nc.scalar.copy(
                    ck[:].rearrange("p (b v) -> p b v", b=QB), pq_view
                )
                if running is None:
                    running = ck
                else:
                    nxt = cols_pool.tile([128, QB * VT], dt.float16, tag="run")
                    nc.vector.tensor_tensor(
                        out=nxt[:], in0=running[:], in1=ck[:],
                        op=mybir.AluOpType.min,
                    )
                    running = nxt
            nc.vector.tensor_reduce(
                mins[:, t : t + 1],
                running[:],
                axis=mybir.AxisListType.X,
                op=mybir.AluOpType.min,
            )

        masked = const.tile([128, T], dt.float32)
        nc.vector.tensor_mul(masked[:], mins[:], msk_sb[:])
        col = const.tile([128, 1], dt.float32)
        nc.vector.tensor_reduce(
            col[:], masked[:], axis=mybir.AxisListType.X, op=mybir.AluOpType.add
        )
        nc.sync.dma_start(out=out, in_=col[:])

    nc.compile()
    return nc


def _prep_core_inputs(verts_b, coords_b, m_b, S):
    """Host-side layout prep for one batch element / core.

    verts_b  [NV, 3] fp32, coords_b [na, 3] fp32 (compacted samples),
    m_b [na] fp32 mask values.  Returns the DRAM input map.
    """
    T = S // 128
    na = coords_b.shape[0]

    bpad = np.zeros((S, 3), dtype=np.float32)
    bpad[:na] = coords_b
    mpad = np.zeros((S,), dtype=np.float32)
    mpad[:na] = m_b
    sqb = np.sum(bpad * bpad, axis=-1, dtype=np.float32)

    b_parts = _bf16_split3(bpad)  # each [S, 3]

    w = (-2.0 * verts_b).astype(np.float32)  # [NV, 3]
    sqv = np.sum(verts_b * verts_b, axis=-1, dtype=np.float32)  # [NV]
    w_parts = _bf16_split3(w)
    s_parts = _bf16_split3(sqv)

    lhsT = np.empty((K, S), dtype=_BF16)
    rhs = np.empty((K, NV), dtype=_BF16)
    for d in range(3):
        for r, (i, j) in enumerate(_PAIRS):
            lhsT[6 * d + r] = b_parts[i][:, d]
            rhs[6 * d + r] = w_parts[j][:, d]
    for j in range(3):
        lhsT[18 + j] = np.ones((S,), dtype=_BF16)
        rhs[18 + j] = s_parts[j]
    sqb_parts = _bf16_split3(sqb)
    for j in range(3):
        lhsT[21 + j] = sqb_parts[j]
        rhs[21 + j] = np.ones((NV,), dtype=_BF16)

    return {
        "lhsT": np.ascontiguousarray(lhsT),
        "rhs": np.ascontiguousarray(rhs),
        "msk": np.ascontiguousarray(mpad.reshape(T, 128).T),
    }


def _prepare_all(verts, bds, indices):
    verts = np.asarray(verts, dtype=np.float32)
    bds = np.asarray(bds, dtype=np.float32)
    idx = np.asarray(indices).astype(np.int64)

    bsel = bds[:, idx, :]  # [BT, NS, 4]
    coords = bsel[..., :3]
    m = bsel[..., 3]

    active = [np.nonzero(m[b] != 0.0)[0] for b in range(BT)]
    max_act = max(len(a) for a in active)
    if max_act == 0:
        return None, None
    S = ((max_act + 127) // 128) * 128

    in_maps = [
        _prep_core_inputs(verts[b], coords[b][active[b]], m[b][active[b]], S)
        for b in range(BT)
    ]
    return S, in_maps


def _ensure_ntff_hook():
    """Register the NTFF profile hook bass_utils expects under axon.

    This container's ``antenv`` lacks ``axon_hooks``; build the equivalent
    from the boot helper so trace=True can capture neuron-profile output.
    Only used by the local test harness (BOUNDARIES_TRACE=1).
    """
    import types

    try:
        from antenv.axon_hooks import get_axon_ntff_profile_hook  # noqa: F401

        return True
    except ImportError:
        pass
    try:
        import antenv
        from trn_agent_boot.trn_boot import _ntff_profile_via_ctypes

        hook = _ntff_profile_via_ctypes("/opt/axon/libaxon_pjrt.so")
        if hook is None:
            return False
        mod = types.ModuleType("antenv.axon_hooks")
        mod.get_axon_ntff_profile_hook = lambda: hook
        mod.set_axon_ntff_profile_hook = lambda h: None
        sys.modules["antenv.axon_hooks"] = mod
        antenv.axon_hooks = mod
        return True
    except Exception:
        return False


def kernel(verts, bds, pix_to_face, indices):
    global _LAST_EXEC_NS
    S, in_maps = _prepare_all(verts, bds, indices)
    if S is None:
        return np.float32(0.0)

    if S not in _COMPILED:
        _COMPILED[S] = _build_program(S)
    nc = _COMPILED[S]

    from concourse import bass_utils

    trace = os.environ.get("BOUNDARIES_TRACE", "0") == "1" and _ensure_ntff_hook()
    if trace:
        # Local profiling only: skip the artifact-bucket upload.
        bass_utils.upload_artifacts = lambda tmpdir: "local://unused"

    try:
        res = bass_utils.run_bass_kernel_spmd(
            nc, in_maps, core_ids=list(range(BT)), trace=trace
        )
    except Exception:
        if not trace:
            raise
        res = bass_utils.run_bass_kernel_spmd(
            nc, in_maps, core_ids=list(range(BT)), trace=False
        )
    _LAST_EXEC_NS = res.exec_time_ns

    total = sum(
        float(np.sum(res.results[b]["out"].astype(np.float64))) for b in range(BT)
    )
    return np.float32(total / (NS * BT))


if __name__ == "__main__":
    # Quick self-check against a local numpy reference on random data.
    rng = np.random.default_rng(0)
    verts = rng.standard_normal((BT, NV, 3), dtype=np.float32)
    bds = rng.standard_normal((BT, NB, 4), dtype=np.float32)
    bds[..., 3] = (rng.random((BT, NB)) > 0.5).astype(np.float32)
    pix = np.zeros((BT, 256, 256, 1), dtype=np.int32)
    idx = rng.permutation(NB)[:NS].astype(np.int64)

    bv = bds[:, idx, :3]
    bm = bds[:, idx, 3]
    d = (
        np.sum(bv * bv, -1)[:, :, None]
        + np.sum(verts * verts, -1)[:, None, :]
        - 2.0 * np.einsum("bsd,bvd->bsv", bv, verts)
    )
    expected = np.mean(np.min(d, -1) * bm)

    actual = kernel(verts, bds, pix, idx)
    rel = abs(actual - expected) / max(abs(expected), 1e-12)
    print(f"expected={expected:.8f} actual={actual:.8f} rel={rel:.3e}")

